# revision 55
# baseline (speedup 1.0000x reference)
"""Trainium2 Bass kernel for a 2-layer LSTM seq2seq (CharSeq2Seq).

Data-parallel over batch: B=2048 split across 8 NeuronCores (256 rows each).
On-device layout is feature-major ("transposed"): states h/c are stored as
[128 (feature chunk), batch] tiles so the recurrent matmul needs no on-device
transposes anywhere:

    gates^T[m_gate, b] = sum_k W^T[k_feat, m_gate].T @ x_or_h^T[k_feat, b]

Matmul inputs are bf16 (fp32 PSUM accumulation); the cell state c stays fp32.
The embedding is folded into the layer-0 input weights on the host
(x @ W_ih^T == onehot @ (emb @ W_ih^T)), so layer 0 consumes the host-built
one-hot [37, S*B_local] directly as its x operand with a single 37-row
contraction chunk. Per step, matmuls are emitted k-major across j-pairs so
the previous step's h_k consumers sit late in the in-order PE stream.

Two gates share each PSUM bank (one [128, 512] f32 bank per gate-pair): only
the bank's first matmul carries start=True (clears has_written bank-wide) and
only its last carries stop=True; the second gate's first write relies on the
per-element has_written bits to overwrite-not-accumulate. A j-group therefore
holds 2 banks instead of 4, doubling the PE's PSUM runway ahead of the
scalar-engine drain (fewer PE micro-stalls / HAM re-throttle exposure);
measured ~6% faster than the 4-banks-per-j layout, bit-identical output.

Each j-group's x-part matmuls are emitted two groups (three in non-FC
phases; all 8 PSUM banks belong to one pool, the FC projection borrows a
bank only in dec-L1) ahead of its h-part, so every step opens with
hazard-free matmuls before the first consumer of h(t-1) — the previous
step's elementwise tail drains under them instead of stalling the PE
(-9% measured). The L1 phases prefetch the next step's y0 tile one step
ahead (y pool bufs=3; with 2 the prefetch DMA head-of-line-blocks the
h spills behind it).

Weight DMAs ride the scalar-engine queue (sync keeps y spills/prefetches),
and each phase's leading weight tiles are emitted before the previous
phase's body so they stream in while it computes (W_BUFS=18 > 2 phases'
16: the early/late split per phase is derived from the pool-ring positions
so every hoisted trigger is provably non-blocking). The gate ACT order is
i, g, f, o so the DVE's i*g multiply starts one ACT slot earlier in the
step-boundary tail.

Measured floor context: 38592 matmuls x 256 cols = cost-model 4.13 ms;
HW runs ~127-137 ns/matmul (LDWEIGHTS exposure / PE p-state), i.e. ~4.9-5.3
ms depending on machine state. fp8 DoubleRow was evaluated and rejected:
e4m3 quantization of W_hh+h pushes logits rel-err to 3-6e-2 (> 2e-2 gate)
even restricted to the L0 layers; row/col tile_position matmuls compile but
fail at runtime in this stack (INTERNAL), so the K=37 one-hot x-part cannot
be dual row-tiled.

Phases per core: enc L0 (spill y0 to DRAM) -> enc L1 -> dec L0 (spill y0)
-> dec L1 + fused FC per step.
"""

import sys

sys.path.insert(0, "/opt/trn_rl_repo")

import numpy as np
import ml_dtypes

import concourse.bass as bass
import concourse.mybir as mybir
import concourse.tile as tile
from concourse import bacc
from concourse.bass_utils import run_bass_kernel_spmd

BF16 = mybir.dt.bfloat16
F32 = mybir.dt.float32

V = 37
E = 256
H = 1024
B = 2048
S = 24
T = 24
SOS = 1
NCORES = 8
BL = B // NCORES          # 256 local batch
NT = S * BL               # 6144 tokens per core
NJ = H // 128             # 8 hidden chunks
NM = 4 * H // 128         # 32 gate chunks
KX0 = E // 128            # 2 x-feature chunks for layer 0
KX1 = H // 128            # 8 x-feature chunks for layer 1

AF = mybir.ActivationFunctionType

XR0 = 64            # L0 x-part contraction rows (one-hot 37 zero-padded to 64)
W_ON_SCALAR = True  # weight DMAs on the scalar-engine queue
W_BUFS = 18         # weight pool buffers (> 16 enables cross-phase prefetch)
TWEAKS = True       # ACT order i,g,f,o + pg-8-bank/depth-3 runway
W_QUEUES = 1        # weight DMA queues: 1=scalar only, 2=scalar+vector

NE0 = 1 + NJ        # weight tiles per L0 phase
NE1 = KX1 + NJ      # weight tiles per L1 phase


def _w_early_counts():
    """Per phase, how many leading weight tiles land on pool buffers whose
    previous occupant belongs to a phase that ended >= one phase ago — i.e.
    tiles whose DMA triggers, emitted before the PREVIOUS phase's body, are
    guaranteed non-blocking and so prefetch during that phase."""
    last_phase: dict = {}
    early = []
    a = 0
    for p, n in enumerate([NE0, NE1, NE0, NE1]):
        e, counting = 0, True
        for i in range(n):
            b = (a + i) % W_BUFS
            ok = last_phase.get(b, -10) <= p - 2
            if counting and ok:
                e += 1
            else:
                counting = False
            last_phase[b] = p
        a += n
        early.append(e)
    return early

_PROG = None  # cached compiled program


def _load_w(nc, wpool, w_dram, kx, lo, hi):
    """Allocate + DMA weight tiles [lo, hi) of a phase's (kx + NJ)-tile list.

    Tile k < kx is an x-part chunk ([XR0, 4H] for L0's padded one-hot);
    tiles kx.. are the NJ hidden chunks of 128 rows.  Weight DMAs ride the
    scalar-engine queue so they never head-of-line-block the sync queue's y
    spills/prefetches; callers emit the next phase's leading tiles EARLY
    (before the previous phase's body) so their triggers execute while that
    phase still runs — the pool semaphores make this safe, and the split
    early/late emission keeps every trigger non-blocking on the ACT queue.
    """
    if kx == 1:
        x_rows = [(0, XR0)]
    else:
        x_rows = [(k * 128, 128) for k in range(kx)]
    row0 = x_rows[-1][0] + x_rows[-1][1]
    wt = []
    for k in range(lo, hi):
        if k < kx:
            r, n = x_rows[k]
        else:
            r, n = row0 + (k - kx) * 128, 128
        # Alternate tiles between the scalar and vector DGE queues so a
        # phase's late tiles stream on two queues at once. Trigger positions
        # (ACT/DVE streams) give the same non-blocking guarantee either way.
        if W_ON_SCALAR:
            eng = nc.scalar if (W_QUEUES == 1 or k % 2 == 0) else nc.vector
        else:
            eng = nc.sync
        w = wpool.tile([n, 4 * H], BF16, tag="w", name="w")
        eng.dma_start(w[:], w_dram[r : r + n, :])
        wt.append(w)
    return wt


def _emit_lstm_phase(
    nc,
    pools,
    consts,
    *,
    wt,             # weight tiles (kx + NJ), built via _load_w
    bias_key,
    kx,
    x_rhs,          # callable (t, k) -> rhs AP for x-part chunk k at step t
    pre_step=None,  # callable (t) -> None, e.g. issue y-tile DMA load
    zero_init,      # True: h0=c0=0 (step 0 skips h-part)
    h_init=None,    # sbuf tile [128, 2048] bf16 when not zero_init
    c_init=None,    # sbuf tile [128, 2048] f32 when not zero_init
    y_out=None,     # DRAM AP [S, 128, 2048] to spill per-step h
    fc=None,        # dict with fcw_sb, fcb_sb, out_dram for fused projection
):
    """One LSTM layer scan over S steps. Returns final (h_tile, c_tile)."""
    hpool = pools["h"]
    cpool = pools["c"]
    gpool = pools["g"]
    spool = pools["s"]
    lpool = pools["l"]
    pg = pools["pg"]
    bias_sb = consts[bias_key]

    h_prev, c_prev = h_init, c_init
    for t in range(S):
        if pre_step is not None:
            pre_step(t)
        first = zero_init and t == 0
        h_new = hpool.tile([128, NJ * BL], BF16, tag="h", name="h")
        c_new = cpool.tile([128, NJ * BL], F32, tag="c", name="c")
        # rhs source per contraction chunk: x-chunks first, then h-chunks.
        # Matmuls are emitted k-major across a j-pair so the h_k(t-1)
        # consumers land late in the PE stream (the prior step's elementwise
        # chain finishes under the x-part instead of stalling the PE).
        nk_step = kx if first else kx + NJ
        # Two gates share one PSUM bank (paired-bank trick, see docstring).
        # The x-part matmuls of each j-group are emitted two (three in
        # non-FC phases — all 8 banks belong to pg there) groups ahead of
        # its h-part, so every step opens with hazard-free matmuls before
        # the first consumer of the previous step's h — runway for the
        # elementwise tail to finish under.
        depth = (2 if fc is not None else 3) if TWEAKS else 2
        ps_by_j = {}

        def open_j(j):
            ps_if = pg.tile([128, 2 * BL], F32, tag="pg", name="pg_if")
            ps_go = pg.tile([128, 2 * BL], F32, tag="pg", name="pg_go")
            ps = [
                ps_if[:, 0:BL],
                ps_if[:, BL : 2 * BL],
                ps_go[:, 0:BL],
                ps_go[:, BL : 2 * BL],
            ]
            ps_by_j[j] = ps
            for ki in range(kx):
                rhs = x_rhs(t, ki)
                for gi in range(4):
                    m = gi * NJ + j
                    nc.tensor.matmul(
                        ps[gi],
                        wt[ki][:, m * 128 : (m + 1) * 128],
                        rhs,
                        start=(ki == 0 and gi in (0, 2)),
                        stop=(first and ki == kx - 1 and gi in (1, 3)),
                        skip_group_check=True,
                    )

        for jj in range(depth):
            open_j(jj)
        for j in range(NJ):
            ps = ps_by_j.pop(j)
            if not first:
                for ki in range(kx, nk_step):
                    k = ki - kx
                    rhs = h_prev[:, k * BL : (k + 1) * BL]
                    for gi in range(4):  # i, f, g, o
                        m = gi * NJ + j
                        nc.tensor.matmul(
                            ps[gi],
                            wt[ki][:, m * 128 : (m + 1) * 128],
                            rhs,
                            start=False,
                            stop=(ki == nk_step - 1 and gi in (1, 3)),
                            skip_group_check=True,
                        )
            if j + depth < NJ:
                open_j(j + depth)
            gs = gpool.tile([128, 4 * BL], F32, tag="g", name="g")
            # ACT order i, g, f, o: the DVE chain needs i*g first, so putting
            # g second lets sc = i*g start one ACT slot earlier — shortens the
            # step-boundary tail behind the last j-group's bank drain.
            order = (
                ((0, AF.Sigmoid), (2, AF.Tanh), (1, AF.Sigmoid), (3, AF.Sigmoid))
                if TWEAKS else
                ((0, AF.Sigmoid), (1, AF.Sigmoid), (2, AF.Tanh), (3, AF.Sigmoid))
            )
            for gi, func in order:
                nc.scalar.activation(
                    gs[:, gi * BL : (gi + 1) * BL],
                    ps[gi],
                    func,
                    bias=bias_sb[:, gi * NJ + j : gi * NJ + j + 1],
                    scale=1.0,
                )
            jsl = slice(j * BL, (j + 1) * BL)
            i_ap = gs[:, 0:BL]
            f_ap = gs[:, BL : 2 * BL]
            g_ap = gs[:, 2 * BL : 3 * BL]
            o_ap = gs[:, 3 * BL : 4 * BL]
            sc = spool.tile([128, BL], BF16, tag="sc", name="sc")
            if first:
                nc.vector.tensor_mul(c_new[:, jsl], i_ap, g_ap)
            else:
                nc.vector.tensor_mul(sc[:], i_ap, g_ap)
                nc.vector.tensor_mul(c_new[:, jsl], f_ap, c_prev[:, jsl])
                nc.vector.tensor_add(c_new[:, jsl], c_new[:, jsl], sc[:])
            nc.scalar.activation(sc[:], c_new[:, jsl], AF.Tanh)
            nc.vector.tensor_mul(h_new[:, jsl], o_ap, sc[:])
        if y_out is not None:
            nc.sync.dma_start(y_out[t], h_new[:])
        if fc is not None:
            # FC PSUM comes from the shared pg pool (one [128, 512] bank),
            # sliced to the [V, BL] the projection needs.
            psf_bank = pg.tile([128, 2 * BL], F32, tag="pg", name="psf")
            psf = psf_bank[0:V, 0:BL]
            for k in range(NJ):
                nc.tensor.matmul(
                    psf[:],
                    fc["fcw_sb"][:, k * V : (k + 1) * V],
                    h_new[:, k * BL : (k + 1) * BL],
                    start=(k == 0),
                    stop=(k == NJ - 1),
                )
            lt = lpool.tile([V, BL], F32, tag="l", name="l")
            nc.vector.tensor_scalar_add(lt[:], psf[:], fc["fcb_sb"][:])
            nc.sync.dma_start(
                fc["out_dram"][:, t * BL : (t + 1) * BL], lt[:]
            )
        h_prev, c_prev = h_new, c_new
    return h_prev, c_prev


def _build_program(reps=1):
    """reps>1 repeats the whole pipeline in one program (timing harness)."""
    nc = bacc.Bacc("TRN2", target_bir_lowering=False, num_devices=1)

    din = lambda name, shape, dt: nc.dram_tensor(
        name, shape, dt, kind="ExternalInput"
    ).ap()
    dint = lambda name, shape, dt: nc.dram_tensor(
        name, shape, dt, kind="Internal"
    ).ap()

    oh_src = din("oh_src", [XR0, NT], BF16)
    oh_dec = din("oh_dec", [XR0, NT], BF16)
    w_e0 = din("w_e0", [XR0 + H, 4 * H], BF16)
    w_e1 = din("w_e1", [KX1 * 128 + H, 4 * H], BF16)
    w_d0 = din("w_d0", [XR0 + H, 4 * H], BF16)
    w_d1 = din("w_d1", [KX1 * 128 + H, 4 * H], BF16)
    b_e0 = din("b_e0", [128, NM], F32)
    b_e1 = din("b_e1", [128, NM], F32)
    b_d0 = din("b_d0", [128, NM], F32)
    b_d1 = din("b_d1", [128, NM], F32)
    fcw = din("fcw", [H, V], BF16)
    fcb = din("fcb", [V, 1], F32)

    y0e = dint("y0e", [S, 128, NJ * BL], BF16)
    y0d = dint("y0d", [S, 128, NJ * BL], BF16)
    c0f = dint("c0f", [128, NJ * BL], F32)
    h1f = dint("h1f", [128, NJ * BL], BF16)
    c1f = dint("c1f", [128, NJ * BL], F32)

    logitsT = nc.dram_tensor(
        "logitsT", [V, NT], F32, kind="ExternalOutput"
    ).ap()

    with tile.TileContext(nc) as tc:
        import contextlib

        with contextlib.ExitStack() as ctx:
            pools = {
                "w": ctx.enter_context(tc.tile_pool(name="w", bufs=W_BUFS)),
                "x": ctx.enter_context(tc.tile_pool(name="x", bufs=1)),
                "y": ctx.enter_context(tc.tile_pool(name="y", bufs=3)),
                "h": ctx.enter_context(tc.tile_pool(name="h", bufs=3)),
                "c": ctx.enter_context(tc.tile_pool(name="c", bufs=2)),
                "g": ctx.enter_context(tc.tile_pool(name="g", bufs=2)),
                "s": ctx.enter_context(tc.tile_pool(name="s", bufs=2)),
                "l": ctx.enter_context(tc.tile_pool(name="l", bufs=1)),
                "const": ctx.enter_context(tc.tile_pool(name="const", bufs=1)),
                "pg": ctx.enter_context(
                    tc.tile_pool(name="pg", bufs=8, space="PSUM")
                ),
            }
            const = pools["const"]
            consts = {}
            for key, drm in (
                ("b_e0", b_e0),
                ("b_e1", b_e1),
                ("b_d0", b_d0),
                ("b_d1", b_d1),
            ):
                consts[key] = const.tile([128, NM], F32, tag=key, name=key)
                nc.sync.dma_start(consts[key][:], drm[:])
            fcw_sb = const.tile([128, NJ * V], BF16, tag="fcw", name="fcw")
            for k in range(NJ):
                nc.sync.dma_start(
                    fcw_sb[:, k * V : (k + 1) * V],
                    fcw[k * 128 : (k + 1) * 128, :],
                )
            fcb_sb = const.tile([V, 1], F32, tag="fcb", name="fcb")
            nc.sync.dma_start(fcb_sb[:], fcb[:])

            drams = dict(w_e0=w_e0, w_e1=w_e1, w_d0=w_d0, w_d1=w_d1)
            for _rep in range(reps):
                _emit_pipeline(
                    nc, pools, consts, fcw_sb, fcb_sb, drams,
                    oh_src, oh_dec, y0e, y0d, c0f, h1f, c1f, logitsT,
                )

    nc.compile()
    return nc


def _emit_pipeline(
    nc, pools, consts, fcw_sb, fcb_sb, drams,
    oh_src, oh_dec, y0e, y0d, c0f, h1f, c1f, logitsT,
):
    if True:
        if True:
            wp = pools["w"]
            # Weight-load schedule: each phase's leading tiles are emitted
            # BEFORE the previous phase's body, so their DMA triggers run
            # while the previous phase computes. The early counts are exactly
            # the allocations landing on buffers already free at that point
            # (see _w_early_counts); the remaining (late) tiles are emitted
            # at their own phase start, where their buffers have just been
            # released — every trigger is non-blocking on the ACT queue.
            _, e1_early, d0_early, d1_early = _w_early_counts()
            # Load step 0's one-hot slice first so the very first x-part
            # matmul can start while the rest of the one-hot streams in.
            ohs = pools["x"].tile([XR0, NT], BF16, tag="x", name="ohs")
            nc.sync.dma_start(ohs[:, 0:BL], oh_src[:, 0:BL])
            nc.sync.dma_start(ohs[:, BL:], oh_src[:, BL:])
            wt_e0 = _load_w(nc, wp, drams["w_e0"], 1, 0, NE0)
            wt_e1 = _load_w(nc, wp, drams["w_e1"], KX1, 0, e1_early)

            # ---- encoder L0 (embedding folded into W: onehot is the x) ----
            h, c = _emit_lstm_phase(
                nc, pools, consts,
                wt=wt_e0, bias_key="b_e0", kx=1,
                x_rhs=lambda t, k: ohs[:, t * BL : (t + 1) * BL],
                zero_init=True,
                y_out=y0e,
            )
            nc.sync.dma_start(c0f[:], c[:])

            # late e1 tiles: their bufs (e0's leading tiles) free at e0 end
            wt_e1 += _load_w(nc, wp, drams["w_e1"], KX1, e1_early, NE1)
            # early d0 tiles: land on e0's trailing bufs, also free at e0 end
            wt_d0 = _load_w(nc, wp, drams["w_d0"], 1, 0, d0_early)
            # decoder one-hot: its single x-ring buf frees at e0 end, so the
            # load overlaps e1. (The h/c init tiles canNOT be hoisted — they
            # share the per-step h/c rings and an early slot would be reused
            # mid-phase by a step tile, deadlocking the ring.)
            ohd = pools["x"].tile([XR0, NT], BF16, tag="x", name="ohd")
            nc.sync.dma_start(ohd[:], oh_dec[:])

            # ---- encoder L1 (streams y0e back per step, 1-step prefetch) ----
            ycur = {}

            def pre_e1(t):
                if t == 0:
                    a = pools["y"].tile([128, NJ * BL], BF16, tag="y", name="y")
                    nc.sync.dma_start(a[:], y0e[0])
                    b = pools["y"].tile([128, NJ * BL], BF16, tag="y", name="y")
                    nc.sync.dma_start(b[:], y0e[1])
                    ycur["t"], ycur["n"] = a, b
                else:
                    ycur["t"] = ycur["n"]
                    if t + 1 < S:
                        nx = pools["y"].tile(
                            [128, NJ * BL], BF16, tag="y", name="y"
                        )
                        nc.sync.dma_start(nx[:], y0e[t + 1])
                        ycur["n"] = nx

            h, c = _emit_lstm_phase(
                nc, pools, consts,
                wt=wt_e1, bias_key="b_e1", kx=KX1,
                x_rhs=lambda t, k: ycur["t"][:, k * BL : (k + 1) * BL],
                pre_step=pre_e1,
                zero_init=True,
            )
            nc.sync.dma_start(h1f[:], h[:])
            nc.sync.dma_start(c1f[:], c[:])

            # late d0 tiles (bufs held by e1's x-chunks until e1 end)
            wt_d0 += _load_w(nc, wp, drams["w_d0"], 1, d0_early, NE0)
            # early d1 tiles: bufs held by e1's trailing tiles, free at e1 end
            wt_d1 = _load_w(nc, wp, drams["w_d1"], KX1, 0, d1_early)

            # ---- decoder L0 ----
            h0i = pools["h"].tile([128, NJ * BL], BF16, tag="h", name="h_init")
            nc.sync.dma_start(h0i[:], y0e[S - 1])
            c0i = pools["c"].tile([128, NJ * BL], F32, tag="c", name="c_init")
            nc.sync.dma_start(c0i[:], c0f[:])
            h, c = _emit_lstm_phase(
                nc, pools, consts,
                wt=wt_d0, bias_key="b_d0", kx=1,
                x_rhs=lambda t, k: ohd[:, t * BL : (t + 1) * BL],
                zero_init=False, h_init=h0i, c_init=c0i,
                y_out=y0d,
            )

            # late d1 tiles (bufs held by d0's tiles until d0 end)
            wt_d1 += _load_w(nc, wp, drams["w_d1"], KX1, d1_early, NE1)

            # ---- decoder L1 + fused FC ----
            h1i = pools["h"].tile([128, NJ * BL], BF16, tag="h", name="h_init")
            nc.sync.dma_start(h1i[:], h1f[:])
            c1i = pools["c"].tile([128, NJ * BL], F32, tag="c", name="c_init")
            nc.sync.dma_start(c1i[:], c1f[:])

            def pre_d1(t):
                if t == 0:
                    a = pools["y"].tile([128, NJ * BL], BF16, tag="y", name="y")
                    nc.sync.dma_start(a[:], y0d[0])
                    b = pools["y"].tile([128, NJ * BL], BF16, tag="y", name="y")
                    nc.sync.dma_start(b[:], y0d[1])
                    ycur["t"], ycur["n"] = a, b
                else:
                    ycur["t"] = ycur["n"]
                    if t + 1 < S:
                        nx = pools["y"].tile(
                            [128, NJ * BL], BF16, tag="y", name="y"
                        )
                        nc.sync.dma_start(nx[:], y0d[t + 1])
                        ycur["n"] = nx

            _emit_lstm_phase(
                nc, pools, consts,
                wt=wt_d1, bias_key="b_d1", kx=KX1,
                x_rhs=lambda t, k: ycur["t"][:, k * BL : (k + 1) * BL],
                pre_step=pre_d1,
                zero_init=False, h_init=h1i, c_init=c1i,
                fc={"fcw_sb": fcw_sb, "fcb_sb": fcb_sb, "out_dram": logitsT},
            )


def _get_program():
    global _PROG
    if _PROG is None:
        _PROG = _build_program()
    return _PROG


def _bf16(a):
    return np.asarray(a, dtype=np.float32).astype(ml_dtypes.bfloat16)


def _prep_shared(inputs):
    emb = np.asarray(inputs["emb"], np.float32)  # [37, 256]
    shared = {}
    for pre, ih, hh, bi, bh in (
        ("e0", "eW_ih0", "eW_hh0", "eb_ih0", "eb_hh0"),
        ("e1", "eW_ih1", "eW_hh1", "eb_ih1", "eb_hh1"),
        ("d0", "dW_ih0", "dW_hh0", "db_ih0", "db_hh0"),
        ("d1", "dW_ih1", "dW_hh1", "db_ih1", "db_hh1"),
    ):
        wih = np.asarray(inputs[ih], np.float32)
        whh = np.asarray(inputs[hh], np.float32)
        if pre in ("e0", "d0"):
            # fold the embedding: x @ W_ih^T == onehot @ (emb @ W_ih^T);
            # zero-pad 37 -> XR0 rows for the 64-aligned dual row-tiling
            xpart = np.zeros((XR0, 4 * H), np.float32)
            xpart[:V] = emb @ wih.T  # [V, 4H] fp32
        else:
            xpart = wih.T
        wt = np.concatenate([xpart, whh.T], axis=0)
        shared[f"w_{pre}"] = np.ascontiguousarray(wt).astype(ml_dtypes.bfloat16)
        b = (
            np.asarray(inputs[bi], np.float32)
            + np.asarray(inputs[bh], np.float32)
        )
        shared[f"b_{pre}"] = np.ascontiguousarray(b.reshape(NM, 128).T)
    shared["fcw"] = np.ascontiguousarray(
        np.asarray(inputs["fcW"], np.float32).T
    ).astype(ml_dtypes.bfloat16)
    shared["fcb"] = np.ascontiguousarray(
        np.asarray(inputs["fcb"], np.float32).reshape(V, 1)
    )
    return shared


def _onehot(tokens_local):
    # tokens_local: [BL, S] int -> one-hot [XR0, S*BL] with col = t*BL + b
    # (rows V..XR0-1 are zero padding for the 64-aligned dual row-tiling)
    flat = np.asarray(tokens_local).T.reshape(-1)  # [S*BL], t-major
    oh = (flat[None, :] == np.arange(XR0)[:, None])
    return np.ascontiguousarray(oh).astype(ml_dtypes.bfloat16)


def kernel(**inputs):
    nc = _get_program()
    shared = _prep_shared(inputs)
    src = np.asarray(inputs["src"])
    tgt = np.asarray(inputs["tgt"])
    dec = np.concatenate(
        [np.full((B, 1), SOS, dtype=tgt.dtype), tgt[:, :-1]], axis=1
    )
    in_maps = []
    for i in range(NCORES):
        sl = slice(i * BL, (i + 1) * BL)
        m = dict(shared)
        m["oh_src"] = _onehot(src[sl])
        m["oh_dec"] = _onehot(dec[sl])
        in_maps.append(m)
    res = None
    for attempt in range(3):
        try:
            res = run_bass_kernel_spmd(
                nc, in_maps, core_ids=list(range(NCORES))
            )
            break
        except Exception:
            if attempt == 2:
                raise
    out = np.empty((B, T, V), np.float32)
    for i in range(NCORES):
        lt = res.results[i]["logitsT"]  # [37, T*BL]
        out[i * BL : (i + 1) * BL] = lt.reshape(V, T, BL).transpose(2, 1, 0)
    return out


if __name__ == "__main__":
    prog = _get_program()
    print("program built OK")



# revision 61
# speedup vs baseline: 1.0477x; 1.0477x over previous
"""Trainium2 Bass kernel for a 2-layer LSTM seq2seq (CharSeq2Seq).

Data-parallel over batch: B=2048 split across 8 NeuronCores (256 rows each).
On-device layout is feature-major ("transposed"): states h/c are stored as
[128 (feature chunk), batch] tiles so the recurrent matmul needs no on-device
transposes anywhere:

    gates^T[m_gate, b] = sum_k W^T[k_feat, m_gate].T @ x_or_h^T[k_feat, b]

Matmul inputs are bf16 (fp32 PSUM accumulation); the cell state c stays fp32.
The embedding is folded into the layer-0 input weights on the host
(x @ W_ih^T == onehot @ (emb @ W_ih^T)), so layer 0 consumes the host-built
one-hot [37, S*B_local] directly as its x operand with a single 37-row
contraction chunk. Per step, matmuls are emitted k-major across j-pairs so
the previous step's h_k consumers sit late in the in-order PE stream.

Two gates share each PSUM bank (one [128, 512] f32 bank per gate-pair): only
the bank's first matmul carries start=True (clears has_written bank-wide) and
only its last carries stop=True; the second gate's first write relies on the
per-element has_written bits to overwrite-not-accumulate. A j-group therefore
holds 2 banks instead of 4, doubling the PE's PSUM runway ahead of the
scalar-engine drain (fewer PE micro-stalls / HAM re-throttle exposure);
measured ~6% faster than the 4-banks-per-j layout, bit-identical output.

Each j-group's x-part matmuls are emitted two groups (three in non-FC
phases; all 8 PSUM banks belong to one pool, the FC projection borrows a
bank only in dec-L1) ahead of its h-part, so every step opens with
hazard-free matmuls before the first consumer of h(t-1) — the previous
step's elementwise tail drains under them instead of stalling the PE
(-9% measured). The L1 phases prefetch the next step's y0 tile one step
ahead (y pool bufs=3; with 2 the prefetch DMA head-of-line-blocks the
h spills behind it).

Weight DMAs ride the scalar-engine queue (sync keeps y spills/prefetches),
and each phase's leading weight tiles are emitted before the previous
phase's body so they stream in while it computes (W_BUFS=18 > 2 phases'
16: the early/late split per phase is derived from the pool-ring positions
so every hoisted trigger is provably non-blocking). The gate ACT order is
i, g, f, o so the DVE's i*g multiply starts one ACT slot earlier in the
step-boundary tail.

Measured floor context: 38592 matmuls x 256 cols = cost-model 4.13 ms;
HW runs ~127-137 ns/matmul (LDWEIGHTS exposure / PE p-state), i.e. ~4.9-5.3
ms depending on machine state. fp8 DoubleRow was evaluated and rejected:
e4m3 quantization of W_hh+h pushes logits rel-err to 3-6e-2 (> 2e-2 gate)
even restricted to the L0 layers; row/col tile_position matmuls compile but
fail at runtime in this stack (INTERNAL), so the K=37 one-hot x-part cannot
be dual row-tiled.

Phases per core: enc L0 (spill y0 to DRAM) -> enc L1 -> dec L0 (spill y0)
-> dec L1 + fused FC per step.
"""

import sys

sys.path.insert(0, "/opt/trn_rl_repo")

import numpy as np
import ml_dtypes

import concourse.bass as bass
import concourse.mybir as mybir
import concourse.tile as tile
from concourse import bacc
from concourse.bass_utils import run_bass_kernel_spmd

BF16 = mybir.dt.bfloat16
F32 = mybir.dt.float32

V = 37
E = 256
H = 1024
B = 2048
S = 24
T = 24
SOS = 1
NCORES = 8
BL = B // NCORES          # 256 local batch
NT = S * BL               # 6144 tokens per core
NJ = H // 128             # 8 hidden chunks
NM = 4 * H // 128         # 32 gate chunks
KX0 = E // 128            # 2 x-feature chunks for layer 0
KX1 = H // 128            # 8 x-feature chunks for layer 1

AF = mybir.ActivationFunctionType

XR0 = 64            # L0 x-part contraction rows (one-hot 37 zero-padded to 64)
W_ON_SCALAR = True  # weight DMAs on the scalar-engine queue
W_BUFS = 18         # weight pool buffers (> 16 enables cross-phase prefetch)
TWEAKS = True       # ACT order i,g,f,o + pg-8-bank/depth-3 runway
W_QUEUES = 1        # weight DMA queues: 1=scalar only, 2=scalar+vector

NE0 = 1 + NJ        # weight tiles per L0 phase
NE1 = KX1 + NJ      # weight tiles per L1 phase


def _w_early_counts():
    """Per phase, how many leading weight tiles land on pool buffers whose
    previous occupant belongs to a phase that ended >= one phase ago — i.e.
    tiles whose DMA triggers, emitted before the PREVIOUS phase's body, are
    guaranteed non-blocking and so prefetch during that phase."""
    last_phase: dict = {}
    early = []
    a = 0
    for p, n in enumerate([NE0, NE1, NE0, NE1]):
        e, counting = 0, True
        for i in range(n):
            b = (a + i) % W_BUFS
            ok = last_phase.get(b, -10) <= p - 2
            if counting and ok:
                e += 1
            else:
                counting = False
            last_phase[b] = p
        a += n
        early.append(e)
    return early

_PROG = None  # cached compiled program


def _load_w(nc, wpool, w_dram, kx, lo, hi):
    """Allocate + DMA weight tiles [lo, hi) of a phase's (kx + NJ)-tile list.

    Tile k < kx is an x-part chunk ([XR0, 4H] for L0's padded one-hot);
    tiles kx.. are the NJ hidden chunks of 128 rows.  Weight DMAs ride the
    scalar-engine queue so they never head-of-line-block the sync queue's y
    spills/prefetches; callers emit the next phase's leading tiles EARLY
    (before the previous phase's body) so their triggers execute while that
    phase still runs — the pool semaphores make this safe, and the split
    early/late emission keeps every trigger non-blocking on the ACT queue.
    """
    if kx == 1:
        x_rows = [(0, XR0)]
    else:
        x_rows = [(k * 128, 128) for k in range(kx)]
    row0 = x_rows[-1][0] + x_rows[-1][1]
    wt = []
    for k in range(lo, hi):
        if k < kx:
            r, n = x_rows[k]
        else:
            r, n = row0 + (k - kx) * 128, 128
        # Alternate tiles between the scalar and vector DGE queues so a
        # phase's late tiles stream on two queues at once. Trigger positions
        # (ACT/DVE streams) give the same non-blocking guarantee either way.
        if W_ON_SCALAR:
            eng = nc.scalar if (W_QUEUES == 1 or k % 2 == 0) else nc.vector
        else:
            eng = nc.sync
        w = wpool.tile([n, 4 * H], BF16, tag="w", name="w")
        eng.dma_start(w[:], w_dram[r : r + n, :])
        wt.append(w)
    return wt


def _emit_lstm_phase(
    nc,
    pools,
    consts,
    *,
    wt,             # weight tiles (kx + NJ), built via _load_w
    bias_key,
    kx,
    x_rhs,          # callable (t, k) -> rhs AP for x-part chunk k at step t
    pre_step=None,  # callable (t) -> None, e.g. issue y-tile DMA load
    post_step=None, # dict {t: callable} run after step t's emission (e.g.
                    # hoist the NEXT phase's first y prefetches so its x-part
                    # matmuls can start under this phase's elementwise tail)
    zero_init,      # True: h0=c0=0 (step 0 skips h-part)
    h_init=None,    # sbuf tile [128, 2048] bf16 when not zero_init
    c_init=None,    # sbuf tile [128, 2048] f32 when not zero_init
    y_out=None,     # DRAM AP [S, 128, 2048] to spill per-step h
    fc=None,        # dict with fcw_sb, fcb_sb, out_dram for fused projection
):
    """One LSTM layer scan over S steps. Returns final (h_tile, c_tile)."""
    hpool = pools["h"]
    cpool = pools["c"]
    gpool = pools["g"]
    spool = pools["s"]
    lpool = pools["l"]
    pg = pools["pg"]
    bias_sb = consts[bias_key]

    h_prev, c_prev = h_init, c_init
    for t in range(S):
        if pre_step is not None:
            pre_step(t)
        first = zero_init and t == 0
        h_new = hpool.tile([128, NJ * BL], BF16, tag="h", name="h")
        c_new = cpool.tile([128, NJ * BL], F32, tag="c", name="c")
        # rhs source per contraction chunk: x-chunks first, then h-chunks.
        # Matmuls are emitted k-major across a j-pair so the h_k(t-1)
        # consumers land late in the PE stream (the prior step's elementwise
        # chain finishes under the x-part instead of stalling the PE).
        nk_step = kx if first else kx + NJ
        # Two gates share one PSUM bank (paired-bank trick, see docstring).
        # The x-part matmuls of each j-group are emitted two (three in
        # non-FC phases — all 8 banks belong to pg there) groups ahead of
        # its h-part, so every step opens with hazard-free matmuls before
        # the first consumer of the previous step's h — runway for the
        # elementwise tail to finish under.
        depth = (2 if fc is not None else 3) if TWEAKS else 2
        ps_by_j = {}

        def open_j(j):
            ps_if = pg.tile([128, 2 * BL], F32, tag="pg", name="pg_if")
            ps_go = pg.tile([128, 2 * BL], F32, tag="pg", name="pg_go")
            ps = [
                ps_if[:, 0:BL],
                ps_if[:, BL : 2 * BL],
                ps_go[:, 0:BL],
                ps_go[:, BL : 2 * BL],
            ]
            ps_by_j[j] = ps
            for ki in range(kx):
                rhs = x_rhs(t, ki)
                for gi in range(4):
                    m = gi * NJ + j
                    nc.tensor.matmul(
                        ps[gi],
                        wt[ki][:, m * 128 : (m + 1) * 128],
                        rhs,
                        start=(ki == 0 and gi in (0, 2)),
                        stop=(first and ki == kx - 1 and gi in (1, 3)),
                        skip_group_check=True,
                    )

        for jj in range(depth):
            open_j(jj)
        for j in range(NJ):
            ps = ps_by_j.pop(j)
            if not first:
                for ki in range(kx, nk_step):
                    k = ki - kx
                    rhs = h_prev[:, k * BL : (k + 1) * BL]
                    for gi in range(4):  # i, f, g, o
                        m = gi * NJ + j
                        nc.tensor.matmul(
                            ps[gi],
                            wt[ki][:, m * 128 : (m + 1) * 128],
                            rhs,
                            start=False,
                            stop=(ki == nk_step - 1 and gi in (1, 3)),
                            skip_group_check=True,
                        )
            if j + depth < NJ:
                open_j(j + depth)
            gs = gpool.tile([128, 4 * BL], F32, tag="g", name="g")
            # ACT order i, g, f, o: the DVE chain needs i*g first, so putting
            # g second lets sc = i*g start one ACT slot earlier — shortens the
            # step-boundary tail behind the last j-group's bank drain.
            order = (
                ((0, AF.Sigmoid), (2, AF.Tanh), (1, AF.Sigmoid), (3, AF.Sigmoid))
                if TWEAKS else
                ((0, AF.Sigmoid), (1, AF.Sigmoid), (2, AF.Tanh), (3, AF.Sigmoid))
            )
            for gi, func in order:
                nc.scalar.activation(
                    gs[:, gi * BL : (gi + 1) * BL],
                    ps[gi],
                    func,
                    bias=bias_sb[:, gi * NJ + j : gi * NJ + j + 1],
                    scale=1.0,
                )
            jsl = slice(j * BL, (j + 1) * BL)
            i_ap = gs[:, 0:BL]
            f_ap = gs[:, BL : 2 * BL]
            g_ap = gs[:, 2 * BL : 3 * BL]
            o_ap = gs[:, 3 * BL : 4 * BL]
            sc = spool.tile([128, BL], BF16, tag="sc", name="sc")
            if first:
                nc.vector.tensor_mul(c_new[:, jsl], i_ap, g_ap)
            else:
                nc.vector.tensor_mul(sc[:], i_ap, g_ap)
                nc.vector.tensor_mul(c_new[:, jsl], f_ap, c_prev[:, jsl])
                nc.vector.tensor_add(c_new[:, jsl], c_new[:, jsl], sc[:])
            nc.scalar.activation(sc[:], c_new[:, jsl], AF.Tanh)
            nc.vector.tensor_mul(h_new[:, jsl], o_ap, sc[:])
        if y_out is not None:
            nc.sync.dma_start(y_out[t], h_new[:])
        if fc is not None:
            # FC PSUM comes from the shared pg pool (one [128, 512] bank),
            # sliced to the [V, BL] the projection needs.
            psf_bank = pg.tile([128, 2 * BL], F32, tag="pg", name="psf")
            psf = psf_bank[0:V, 0:BL]
            for k in range(NJ):
                nc.tensor.matmul(
                    psf[:],
                    fc["fcw_sb"][:, k * V : (k + 1) * V],
                    h_new[:, k * BL : (k + 1) * BL],
                    start=(k == 0),
                    stop=(k == NJ - 1),
                )
            lt = lpool.tile([V, BL], F32, tag="l", name="l")
            nc.vector.tensor_scalar_add(lt[:], psf[:], fc["fcb_sb"][:])
            nc.sync.dma_start(
                fc["out_dram"][:, t * BL : (t + 1) * BL], lt[:]
            )
        h_prev, c_prev = h_new, c_new
        if post_step is not None and t in post_step:
            post_step[t]()
    return h_prev, c_prev


def _build_program(reps=1):
    """reps>1 repeats the whole pipeline in one program (timing harness)."""
    nc = bacc.Bacc("TRN2", target_bir_lowering=False, num_devices=1)

    din = lambda name, shape, dt: nc.dram_tensor(
        name, shape, dt, kind="ExternalInput"
    ).ap()
    dint = lambda name, shape, dt: nc.dram_tensor(
        name, shape, dt, kind="Internal"
    ).ap()

    oh_src = din("oh_src", [XR0, NT], BF16)
    oh_dec = din("oh_dec", [XR0, NT], BF16)
    w_e0 = din("w_e0", [XR0 + H, 4 * H], BF16)
    w_e1 = din("w_e1", [KX1 * 128 + H, 4 * H], BF16)
    w_d0 = din("w_d0", [XR0 + H, 4 * H], BF16)
    w_d1 = din("w_d1", [KX1 * 128 + H, 4 * H], BF16)
    b_e0 = din("b_e0", [128, NM], F32)
    b_e1 = din("b_e1", [128, NM], F32)
    b_d0 = din("b_d0", [128, NM], F32)
    b_d1 = din("b_d1", [128, NM], F32)
    fcw = din("fcw", [H, V], BF16)
    fcb = din("fcb", [V, 1], F32)

    y0e = dint("y0e", [S, 128, NJ * BL], BF16)
    y0d = dint("y0d", [S, 128, NJ * BL], BF16)
    c0f = dint("c0f", [128, NJ * BL], F32)
    h1f = dint("h1f", [128, NJ * BL], BF16)
    c1f = dint("c1f", [128, NJ * BL], F32)

    logitsT = nc.dram_tensor(
        "logitsT", [V, NT], F32, kind="ExternalOutput"
    ).ap()

    with tile.TileContext(nc) as tc:
        import contextlib

        with contextlib.ExitStack() as ctx:
            pools = {
                "w": ctx.enter_context(tc.tile_pool(name="w", bufs=W_BUFS)),
                "x": ctx.enter_context(tc.tile_pool(name="x", bufs=1)),
                "y": ctx.enter_context(tc.tile_pool(name="y", bufs=3)),
                "h": ctx.enter_context(tc.tile_pool(name="h", bufs=3)),
                "c": ctx.enter_context(tc.tile_pool(name="c", bufs=2)),
                "g": ctx.enter_context(tc.tile_pool(name="g", bufs=2)),
                "s": ctx.enter_context(tc.tile_pool(name="s", bufs=2)),
                "l": ctx.enter_context(tc.tile_pool(name="l", bufs=1)),
                "const": ctx.enter_context(tc.tile_pool(name="const", bufs=1)),
                "pg": ctx.enter_context(
                    tc.tile_pool(name="pg", bufs=8, space="PSUM")
                ),
            }
            const = pools["const"]
            consts = {}
            for key, drm in (
                ("b_e0", b_e0),
                ("b_e1", b_e1),
                ("b_d0", b_d0),
                ("b_d1", b_d1),
            ):
                consts[key] = const.tile([128, NM], F32, tag=key, name=key)
                nc.sync.dma_start(consts[key][:], drm[:])
            fcw_sb = const.tile([128, NJ * V], BF16, tag="fcw", name="fcw")
            for k in range(NJ):
                nc.sync.dma_start(
                    fcw_sb[:, k * V : (k + 1) * V],
                    fcw[k * 128 : (k + 1) * 128, :],
                )
            fcb_sb = const.tile([V, 1], F32, tag="fcb", name="fcb")
            nc.sync.dma_start(fcb_sb[:], fcb[:])

            drams = dict(w_e0=w_e0, w_e1=w_e1, w_d0=w_d0, w_d1=w_d1)
            for _rep in range(reps):
                _emit_pipeline(
                    nc, pools, consts, fcw_sb, fcb_sb, drams,
                    oh_src, oh_dec, y0e, y0d, c0f, h1f, c1f, logitsT,
                )

    nc.compile()
    return nc


def _emit_pipeline(
    nc, pools, consts, fcw_sb, fcb_sb, drams,
    oh_src, oh_dec, y0e, y0d, c0f, h1f, c1f, logitsT,
):
    if True:
        if True:
            wp = pools["w"]
            # Weight-load schedule: each phase's leading tiles are emitted
            # BEFORE the previous phase's body, so their DMA triggers run
            # while the previous phase computes. The early counts are exactly
            # the allocations landing on buffers already free at that point
            # (see _w_early_counts); the remaining (late) tiles are emitted
            # at their own phase start, where their buffers have just been
            # released — every trigger is non-blocking on the ACT queue.
            _, e1_early, d0_early, d1_early = _w_early_counts()
            # Load step 0's one-hot slice first so the very first x-part
            # matmul can start while the rest of the one-hot streams in.
            ohs = pools["x"].tile([XR0, NT], BF16, tag="x", name="ohs")
            nc.sync.dma_start(ohs[:, 0:BL], oh_src[:, 0:BL])
            nc.sync.dma_start(ohs[:, BL:], oh_src[:, BL:])
            wt_e0 = _load_w(nc, wp, drams["w_e0"], 1, 0, NE0)
            wt_e1 = _load_w(nc, wp, drams["w_e1"], KX1, 0, e1_early)

            # Hoisted y prefetches: after step 2 of an L0 phase, issue the
            # following L1 phase's first two y-tile loads (their source
            # spills are already queued ahead on the sync queue, and the
            # y-ring buffers are free by then), so the L1 x-part matmuls can
            # start under the L0 phase's final elementwise tail.
            ypre = {}

            def hoist_y(y_dram):
                def f():
                    a = pools["y"].tile(
                        [128, NJ * BL], BF16, tag="y", name="y"
                    )
                    nc.sync.dma_start(a[:], y_dram[0])
                    b = pools["y"].tile(
                        [128, NJ * BL], BF16, tag="y", name="y"
                    )
                    nc.sync.dma_start(b[:], y_dram[1])
                    ypre["pair"] = (a, b)
                return f

            # ---- encoder L0 (embedding folded into W: onehot is the x) ----
            h, c = _emit_lstm_phase(
                nc, pools, consts,
                wt=wt_e0, bias_key="b_e0", kx=1,
                x_rhs=lambda t, k: ohs[:, t * BL : (t + 1) * BL],
                post_step={2: hoist_y(y0e)},
                zero_init=True,
                y_out=y0e,
            )
            nc.sync.dma_start(c0f[:], c[:])

            # late e1 tiles: their bufs (e0's leading tiles) free at e0 end
            wt_e1 += _load_w(nc, wp, drams["w_e1"], KX1, e1_early, NE1)
            # early d0 tiles: land on e0's trailing bufs, also free at e0 end
            wt_d0 = _load_w(nc, wp, drams["w_d0"], 1, 0, d0_early)
            # decoder one-hot: its single x-ring buf frees at e0 end, so the
            # load overlaps e1. (The h/c init tiles canNOT be hoisted — they
            # share the per-step h/c rings and an early slot would be reused
            # mid-phase by a step tile, deadlocking the ring.)
            ohd = pools["x"].tile([XR0, NT], BF16, tag="x", name="ohd")
            nc.sync.dma_start(ohd[:], oh_dec[:])

            # ---- encoder L1 (streams y0e back per step, 1-step prefetch) ----
            ycur = {}

            def pre_e1(t):
                if t == 0:
                    ycur["t"], ycur["n"] = ypre.pop("pair")
                else:
                    ycur["t"] = ycur["n"]
                    if t + 1 < S:
                        nx = pools["y"].tile(
                            [128, NJ * BL], BF16, tag="y", name="y"
                        )
                        nc.sync.dma_start(nx[:], y0e[t + 1])
                        ycur["n"] = nx

            h, c = _emit_lstm_phase(
                nc, pools, consts,
                wt=wt_e1, bias_key="b_e1", kx=KX1,
                x_rhs=lambda t, k: ycur["t"][:, k * BL : (k + 1) * BL],
                pre_step=pre_e1,
                zero_init=True,
            )
            nc.sync.dma_start(h1f[:], h[:])
            nc.sync.dma_start(c1f[:], c[:])

            # late d0 tiles (bufs held by e1's x-chunks until e1 end)
            wt_d0 += _load_w(nc, wp, drams["w_d0"], 1, d0_early, NE0)
            # early d1 tiles: bufs held by e1's trailing tiles, free at e1 end
            wt_d1 = _load_w(nc, wp, drams["w_d1"], KX1, 0, d1_early)

            # ---- decoder L0 ----
            h0i = pools["h"].tile([128, NJ * BL], BF16, tag="h", name="h_init")
            nc.sync.dma_start(h0i[:], y0e[S - 1])
            c0i = pools["c"].tile([128, NJ * BL], F32, tag="c", name="c_init")
            nc.sync.dma_start(c0i[:], c0f[:])
            h, c = _emit_lstm_phase(
                nc, pools, consts,
                wt=wt_d0, bias_key="b_d0", kx=1,
                x_rhs=lambda t, k: ohd[:, t * BL : (t + 1) * BL],
                post_step={2: hoist_y(y0d)},
                zero_init=False, h_init=h0i, c_init=c0i,
                y_out=y0d,
            )

            # late d1 tiles (bufs held by d0's tiles until d0 end)
            wt_d1 += _load_w(nc, wp, drams["w_d1"], KX1, d1_early, NE1)

            # ---- decoder L1 + fused FC ----
            h1i = pools["h"].tile([128, NJ * BL], BF16, tag="h", name="h_init")
            nc.sync.dma_start(h1i[:], h1f[:])
            c1i = pools["c"].tile([128, NJ * BL], F32, tag="c", name="c_init")
            nc.sync.dma_start(c1i[:], c1f[:])

            def pre_d1(t):
                if t == 0:
                    ycur["t"], ycur["n"] = ypre.pop("pair")
                else:
                    ycur["t"] = ycur["n"]
                    if t + 1 < S:
                        nx = pools["y"].tile(
                            [128, NJ * BL], BF16, tag="y", name="y"
                        )
                        nc.sync.dma_start(nx[:], y0d[t + 1])
                        ycur["n"] = nx

            _emit_lstm_phase(
                nc, pools, consts,
                wt=wt_d1, bias_key="b_d1", kx=KX1,
                x_rhs=lambda t, k: ycur["t"][:, k * BL : (k + 1) * BL],
                pre_step=pre_d1,
                zero_init=False, h_init=h1i, c_init=c1i,
                fc={"fcw_sb": fcw_sb, "fcb_sb": fcb_sb, "out_dram": logitsT},
            )


def _get_program():
    global _PROG
    if _PROG is None:
        _PROG = _build_program()
    return _PROG


def _bf16(a):
    return np.asarray(a, dtype=np.float32).astype(ml_dtypes.bfloat16)


def _prep_shared(inputs):
    emb = np.asarray(inputs["emb"], np.float32)  # [37, 256]
    shared = {}
    for pre, ih, hh, bi, bh in (
        ("e0", "eW_ih0", "eW_hh0", "eb_ih0", "eb_hh0"),
        ("e1", "eW_ih1", "eW_hh1", "eb_ih1", "eb_hh1"),
        ("d0", "dW_ih0", "dW_hh0", "db_ih0", "db_hh0"),
        ("d1", "dW_ih1", "dW_hh1", "db_ih1", "db_hh1"),
    ):
        wih = np.asarray(inputs[ih], np.float32)
        whh = np.asarray(inputs[hh], np.float32)
        if pre in ("e0", "d0"):
            # fold the embedding: x @ W_ih^T == onehot @ (emb @ W_ih^T);
            # zero-pad 37 -> XR0 rows for the 64-aligned dual row-tiling
            xpart = np.zeros((XR0, 4 * H), np.float32)
            xpart[:V] = emb @ wih.T  # [V, 4H] fp32
        else:
            xpart = wih.T
        wt = np.concatenate([xpart, whh.T], axis=0)
        shared[f"w_{pre}"] = np.ascontiguousarray(wt).astype(ml_dtypes.bfloat16)
        b = (
            np.asarray(inputs[bi], np.float32)
            + np.asarray(inputs[bh], np.float32)
        )
        shared[f"b_{pre}"] = np.ascontiguousarray(b.reshape(NM, 128).T)
    shared["fcw"] = np.ascontiguousarray(
        np.asarray(inputs["fcW"], np.float32).T
    ).astype(ml_dtypes.bfloat16)
    shared["fcb"] = np.ascontiguousarray(
        np.asarray(inputs["fcb"], np.float32).reshape(V, 1)
    )
    return shared


def _onehot(tokens_local):
    # tokens_local: [BL, S] int -> one-hot [XR0, S*BL] with col = t*BL + b
    # (rows V..XR0-1 are zero padding for the 64-aligned dual row-tiling)
    flat = np.asarray(tokens_local).T.reshape(-1)  # [S*BL], t-major
    oh = (flat[None, :] == np.arange(XR0)[:, None])
    return np.ascontiguousarray(oh).astype(ml_dtypes.bfloat16)


def kernel(**inputs):
    nc = _get_program()
    shared = _prep_shared(inputs)
    src = np.asarray(inputs["src"])
    tgt = np.asarray(inputs["tgt"])
    dec = np.concatenate(
        [np.full((B, 1), SOS, dtype=tgt.dtype), tgt[:, :-1]], axis=1
    )
    in_maps = []
    for i in range(NCORES):
        sl = slice(i * BL, (i + 1) * BL)
        m = dict(shared)
        m["oh_src"] = _onehot(src[sl])
        m["oh_dec"] = _onehot(dec[sl])
        in_maps.append(m)
    res = None
    for attempt in range(3):
        try:
            res = run_bass_kernel_spmd(
                nc, in_maps, core_ids=list(range(NCORES))
            )
            break
        except Exception:
            if attempt == 2:
                raise
    out = np.empty((B, T, V), np.float32)
    for i in range(NCORES):
        lt = res.results[i]["logitsT"]  # [37, T*BL]
        out[i * BL : (i + 1) * BL] = lt.reshape(V, T, BL).transpose(2, 1, 0)
    return out


if __name__ == "__main__":
    prog = _get_program()
    print("program built OK")

